# revision 1
# baseline (speedup 1.0000x reference)
"""Trainium2 Bass kernel for nn_AutoGraderPrototypeModel (retrieval_knn).

Computes, for full inputs hidden_states [1024, 256, 1024] f32 and
prototype_weight [512, 1024] f32:

    a      = mean(hidden_states, axis=1)                  # [B, D]
    logits = 2 a @ proto.T - ||a||^2 - ||proto||^2        # [B, 512]
    out    = logits.reshape(B, 64, 8).mean(axis=1)        # [B, 8]

Sharding: data-parallel over batch across 8 NeuronCores (128 batch rows
per core, prototype table replicated). The dominant cost is streaming the
128 MiB hidden_states shard from HBM; the pure-DMA wall measured on this
part is ~349 GB/s/core (384 us), the kernel runs at ~96% of that.

DMA layout: strided partition reads (1 MiB partition stride) only reach
~190 GB/s/core on this part, while fully-linear reads reach ~350 GB/s.
Tiles are therefore loaded as flat contiguous [128, WPP] blocks, 1 MiB
per dma_start, alternating between the two HWDGE rings (SP/ACT) so the
per-tile buffer-free waits and descriptor generation pipeline in
parallel; with an even buffer count each buffer's reuse stays on one
ring, so WAW ordering rides the ring FIFO.

Compute (v2 path): per tile the DVE folds the two in-partition t-rows
into a bf16 partial; a PE matmul with a sliding block-column bf16 mask
(value 1/T) scatter-accumulates each batch's partitions into PSUM
a[128b, 1024d]. The epilogue transposes a in bf16, runs the 8 logits
matmuls in bf16 against the bf16 2*proto.T table, subtracts ||a||^2 as a
per-partition scalar, label-means on the DVE, and adds the label-meaned
-||proto||^2 as a precomputed [128, 8] broadcast (computed once in prep,
f32). bf16 is safe here: logits are dominated by the exactly-computed
-||proto||^2 term (~1024) and the harness gate is 2e-2; measured rel err
is ~6e-6. NOTE: DVE tensor_tensor_reduce hangs this device — keep
use_ttr=False.
"""

import os

os.environ.setdefault("JAX_PLATFORMS", "axon,cpu")

from contextlib import ExitStack

import numpy as np

B, T, D = 1024, 256, 1024
M_PROTO = 512
NUM_LABELS = 8
NUM_PROTOTYPES = 64
N_CORES = 8
BS = B // N_CORES  # 128 batch rows per core
P = 128            # SBUF partitions
WPP = 2048         # words per partition per DMA tile (tile = P*WPP*4 bytes)
HS_BUFS = 10

_cached = {}


def _build_program(reps=1, wpp=WPP, hs_bufs=HS_BUFS, act_pt2=False,
                   ttr_asq=False, stage1=True, split_dma=False,
                   dma_only=False, ring_mode="alt2", v2=True,
                   part_bufs=3, a_bufs=1, pool_dt="bf16", epi_dt="bf16",
                   use_ttr=False, alt_consumer=False):
    import concourse.mybir as mybir
    import concourse.tile as tile
    from concourse import bacc, masks

    f32 = mybir.dt.float32
    bf16 = mybir.dt.bfloat16
    pdt = bf16 if pool_dt == "bf16" else f32
    edt = bf16 if epi_dt == "bf16" else f32
    KD = D // P                      # 8 contraction chunks of 128 over D
    MG = M_PROTO // P                # 4 prototype groups of 128
    words_per_tile = P * wpp
    NT = (BS * T * D) // words_per_tile  # linear tiles per shard
    n_rows = wpp // D                # t-rows per partition (stage-1 depth)
    assert wpp % D == 0
    # batches per tile as a fraction: bpt_num/bpt_den
    bpt_num, bpt_den = words_per_tile, T * D
    n_cols = max(bpt_num // bpt_den, 1)   # mask columns per tile
    grp = P // n_cols if bpt_num >= bpt_den else P

    nc = bacc.Bacc("TRN2", target_bir_lowering=False, debug=False,
                   num_devices=N_CORES)
    hs = nc.dram_tensor("hidden_states", [BS, T, D], f32, kind="ExternalInput").ap()
    pw = nc.dram_tensor("prototype_weight", [M_PROTO, D], f32, kind="ExternalInput").ap()
    out = nc.dram_tensor("out", [BS, NUM_LABELS], f32, kind="ExternalOutput").ap()

    hs_flat = hs.rearrange("b t d -> (b t d)")

    with tile.TileContext(nc) as tc, ExitStack() as ctx:
        hs_pool = ctx.enter_context(tc.tile_pool(name="hs", bufs=hs_bufs))
        part_pool = ctx.enter_context(tc.tile_pool(name="part", bufs=part_bufs))
        work = ctx.enter_context(tc.tile_pool(name="work", bufs=1))
        psum_t = ctx.enter_context(tc.tile_pool(name="psum_t", bufs=2, space="PSUM"))
        psum_a = ctx.enter_context(tc.tile_pool(name="psum_a", bufs=1, space="PSUM"))

        state = {}

        def prep2():
            # bf16 identity for a-transposes; zp mask exact in bf16 (1/256)
            ident = work.tile([P, P], f32, tag="ident", name="ident")
            masks.make_identity(nc, ident[:])
            ident_bf = work.tile([P, P], edt, tag="ident_bf", name="ident_bf")
            masks.make_identity(nc, ident_bf[:])
            ones_m1 = work.tile([P, 1], f32, tag="ones_m1", name="ones_m1")
            nc.gpsimd.memset(ones_m1[:], 1.0)
            ones_k1 = work.tile([1, P], f32, tag="ones_k1", name="ones_k1")
            nc.gpsimd.memset(ones_k1[:], 1.0)

            zp = work.tile([P, 2 * P], pdt, tag="zp", name="zp")
            nc.gpsimd.memset(zp[:], 0.0)
            for c in range(n_cols):
                nc.gpsimd.memset(zp[grp * c:grp * (c + 1), P + c:P + c + 1],
                                 1.0 / T)

            proto_sb = []
            for j in range(MG):
                pj = work.tile([P, D], f32, tag=f"proto{j}", name=f"proto{j}")
                nc.gpsimd.dma_start(pj[:], pw[j * P:(j + 1) * P, :])
                proto_sb.append(pj)

            # protoT2_bf[k] = bf16(2 proto.T chunk); b_sq accumulated in f32
            protoT2 = [work.tile([P, M_PROTO], edt, tag=f"pT2b_{k}",
                                 name=f"pT2b_{k}") for k in range(KD)]
            p2f = work.tile([P, M_PROTO], f32, tag="p2f", name="p2f")
            sqT = work.tile([P, M_PROTO], f32, tag="sqT", name="sqT")
            bsq_ps = psum_a.tile([1, M_PROTO], f32, tag="bsq", name="bsq_ps")
            for k in range(KD):
                for j in range(MG):
                    pt = psum_t.tile([P, P], f32, tag="tp", name="pt", bufs=1)
                    nc.tensor.transpose(pt[:], proto_sb[j][:, k * P:(k + 1) * P],
                                        ident[:])
                    nc.vector.tensor_scalar_mul(
                        p2f[:, j * P:(j + 1) * P], pt[:], 2.0)
                nc.vector.tensor_copy(protoT2[k][:], p2f[:])
                nc.vector.tensor_mul(sqT[:], p2f[:], p2f[:])
                nc.tensor.matmul(bsq_ps[:], ones_m1[:], sqT[:],
                                 start=(k == 0), stop=(k == KD - 1))
            # nbl[l] = -(1/64) sum_p b_sq[p*8+l]; (2p)^2 scale fixed by -0.25
            neg_bsq = work.tile([1, M_PROTO], f32, tag="neg_bsq", name="neg_bsq")
            nc.scalar.mul(neg_bsq[:], bsq_ps[:], -0.25)
            nbl = work.tile([1, NUM_LABELS], f32, tag="nbl", name="nbl")
            nbv = neg_bsq[:].rearrange("a (p l) -> a l p", l=NUM_LABELS)
            nc.vector.tensor_reduce(nbl[:], nbv, axis=mybir.AxisListType.X,
                                    op=mybir.AluOpType.add)
            nc.scalar.mul(nbl[:], nbl[:], 1.0 / NUM_PROTOTYPES)
            ptb = psum_t.tile([P, P], f32, tag="tp", name="ptb", bufs=1)
            nc.tensor.matmul(ptb[:, 0:NUM_LABELS], ones_k1[:], nbl[:],
                             start=True, stop=True)
            nbl_bcast = work.tile([P, NUM_LABELS], f32, tag="nbl_b",
                                  name="nbl_bcast")
            nc.vector.tensor_copy(nbl_bcast[:], ptb[:, 0:NUM_LABELS])

            state.update(ident_bf=ident_bf, zp=zp, protoT2=protoT2,
                         nbl_bcast=nbl_bcast)

        def stream2():
            zp = state["zp"]
            protoT2 = state["protoT2"]
            dma_eng = {"alt2": [nc.sync, nc.scalar],
                       "mono": [nc.sync, nc.sync],
                       "tri": [nc.sync, nc.scalar, nc.gpsimd]}[ring_mode]
            ne = len(dma_eng)

            a_ps = psum_a.tile([P, D], f32, tag="a_ps", name="a_ps",
                               bufs=a_bufs)
            for it in range(NT):
                tl = hs_pool.tile([P, wpp], f32, tag="hs", name="tl")
                src = hs_flat[it * words_per_tile:(it + 1) * words_per_tile]
                s2 = src.rearrange("(p w) -> p w", p=P)
                dma_eng[it % ne].dma_start(tl[:], s2)
                s_i = (it * bpt_num) // bpt_den
                lhsT = zp[:, P - s_i:2 * P - s_i]
                partial = part_pool.tile([P, D], pdt, tag="part",
                                         name="partial")
                # stage-1 adds on DVE, or alternating DVE/Pool per tile
                veng = nc.gpsimd if (alt_consumer and it % 2) else nc.vector
                if n_rows == 2:
                    veng.tensor_add(partial[:], tl[:, 0:D], tl[:, D:2 * D])
                else:
                    # chain f32 adds, final add casts to pool dtype
                    pf = part_pool.tile([P, D], f32, tag="partf", name="pf")
                    veng.tensor_add(pf[:], tl[:, 0:D], tl[:, D:2 * D])
                    for j in range(2, n_rows - 1):
                        veng.tensor_add(pf[:], pf[:],
                                        tl[:, j * D:(j + 1) * D])
                    veng.tensor_add(partial[:], pf[:],
                                    tl[:, (n_rows - 1) * D:n_rows * D])
                for h in range(2):
                    nc.tensor.matmul(a_ps[:, h * 512:(h + 1) * 512], lhsT,
                                     partial[:, h * 512:(h + 1) * 512],
                                     start=(it == 0), stop=(it == NT - 1),
                                     skip_group_check=True)

            # epilogue: a in bf16, logits matmuls in bf16, b_sq folded post-hoc
            a_sb = work.tile([P, D], edt, tag="a", name="a_sb")
            nc.scalar.mul(a_sb[:], a_ps[:], 1.0)

            sq_tmp = work.tile([P, D], f32, tag="sq_tmp", name="sq_tmp")
            asq = work.tile([P, 1], f32, tag="asq", name="asq")
            if use_ttr:
                nc.vector.tensor_tensor_reduce(
                    out=sq_tmp[:], in0=a_sb[:], in1=a_sb[:], scale=1.0,
                    scalar=0.0, op0=mybir.AluOpType.mult,
                    op1=mybir.AluOpType.add, accum_out=asq[:])
            else:
                nc.vector.tensor_mul(sq_tmp[:], a_sb[:], a_sb[:])
                nc.vector.tensor_reduce(asq[:], sq_tmp[:],
                                        axis=mybir.AxisListType.X,
                                        op=mybir.AluOpType.add)

            aTs = []
            for k in range(KD):
                pt = psum_t.tile([P, P], edt, tag="tpb", name="pt")
                nc.tensor.transpose(pt[:], a_sb[:, k * P:(k + 1) * P],
                                    state["ident_bf"][:])
                aT = work.tile([P, P], edt, tag=f"aT{k}", name=f"aT{k}")
                nc.vector.tensor_copy(aT[:], pt[:])
                aTs.append(aT)

            lg_ps = psum_a.tile([P, M_PROTO], f32, tag="lg", name="lg_ps")
            for k in range(KD):
                nc.tensor.matmul(lg_ps[:], aTs[k][:], protoT2[k][:],
                                 start=(k == 0), stop=(k == KD - 1))

            lg_sb = work.tile([P, M_PROTO], f32, tag="lg_sb", name="lg_sb")
            nc.vector.tensor_scalar_sub(lg_sb[:], lg_ps[:], asq[:])

            out_pre = work.tile([P, NUM_LABELS], f32, tag="out_pre",
                                name="out_pre")
            lgv = lg_sb[:].rearrange("b (p l) -> b l p", l=NUM_LABELS)
            nc.vector.tensor_reduce(out_pre[:], lgv, axis=mybir.AxisListType.X,
                                    op=mybir.AluOpType.add)
            nc.scalar.mul(out_pre[:], out_pre[:], 1.0 / NUM_PROTOTYPES)
            out_sb = work.tile([P, NUM_LABELS], f32, tag="out_sb", name="out_sb")
            nc.vector.tensor_add(out_sb[:], out_pre[:], state["nbl_bcast"][:])
            nc.gpsimd.dma_start(out[:, :], out_sb[:])

        def prep():
            ident = work.tile([P, P], f32, tag="ident", name="ident")
            masks.make_identity(nc, ident[:])
            ones_m1 = work.tile([P, 1], f32, tag="ones_m1", name="ones_m1")
            nc.gpsimd.memset(ones_m1[:], 1.0)
            ones_k1 = work.tile([1, P], f32, tag="ones_k1", name="ones_k1")
            nc.gpsimd.memset(ones_k1[:], 1.0)

            # Sliding mask for stage-2 pooling: zp[p, P + c] = 1/T iff
            # c == p // grp (c < n_cols). lhsT for tile i is
            # zp[:, P - s_i : 2P - s_i] with s_i = floor(i * bpt).
            zp = work.tile([P, 2 * P], f32, tag="zp", name="zp")
            nc.gpsimd.memset(zp[:], 0.0)
            for c in range(n_cols):
                nc.gpsimd.memset(zp[grp * c:grp * (c + 1), P + c:P + c + 1],
                                 1.0 / T)

            # protoT2[k] = 2 * proto.T d-chunk; sqT[k] = (2 proto.T)^2
            proto_sb = []
            for j in range(MG):
                pj = work.tile([P, D], f32, tag=f"proto{j}", name=f"proto{j}")
                nc.gpsimd.dma_start(pj[:], pw[j * P:(j + 1) * P, :])
                proto_sb.append(pj)

            protoT2 = [work.tile([P, M_PROTO], f32, tag=f"pT2_{k}", name=f"pT2_{k}")
                       for k in range(KD)]
            sqT = [work.tile([P, M_PROTO], f32, tag=f"sqT_{k}", name=f"sqT_{k}")
                   for k in range(KD)]
            for k in range(KD):
                for j in range(MG):
                    pt = psum_t.tile([P, P], f32, tag="tp", name="pt")
                    nc.tensor.transpose(pt[:], proto_sb[j][:, k * P:(k + 1) * P],
                                        ident[:])
                    if act_pt2:
                        nc.scalar.mul(protoT2[k][:, j * P:(j + 1) * P],
                                      pt[:], 2.0)
                    else:
                        nc.vector.tensor_scalar_mul(
                            protoT2[k][:, j * P:(j + 1) * P], pt[:], 2.0)
                # (2 protoT)^2 = 4 protoT^2; compensated below via -0.25 scale
                nc.vector.tensor_mul(sqT[k][:], protoT2[k][:], protoT2[k][:])

            # b_sq[m] as a [1, 512] row via ones-matmul over squared protoT
            bsq_ps = psum_a.tile([1, M_PROTO], f32, tag="bsq", name="bsq_ps")
            for k in range(KD):
                nc.tensor.matmul(bsq_ps[:], ones_m1[:], sqT[k][:],
                                 start=(k == 0), stop=(k == KD - 1))
            neg_bsq = work.tile([1, M_PROTO], f32, tag="neg_bsq", name="neg_bsq")
            nc.scalar.mul(neg_bsq[:], bsq_ps[:], -0.25)

            state.update(ident=ident, ones_k1=ones_k1, zp=zp, neg_bsq=neg_bsq,
                         protoT2=protoT2)

        def stream_dma_only():
            # BW probe: stream all tiles, no compute consumers (WAW deps on
            # the pool bufs still order reuse); write a dummy output.
            dma_eng = {"alt2": [nc.sync, nc.scalar],
                       "mono": [nc.sync, nc.sync],
                       "tri": [nc.sync, nc.scalar, nc.gpsimd]}[ring_mode]
            ne = len(dma_eng)
            for it in range(NT):
                tl = hs_pool.tile([P, wpp], f32, tag="hs", name="tl")
                src = hs_flat[it * words_per_tile:(it + 1) * words_per_tile]
                s2 = src.rearrange("(p w) -> p w", p=P)
                dma_eng[it % ne].dma_start(tl[:], s2)
            out_sb = work.tile([P, NUM_LABELS], f32, tag="out_sb",
                               name="out_sb")
            nc.gpsimd.memset(out_sb[:], 0.0)
            nc.gpsimd.dma_start(out[:, :], out_sb[:])

        def stream():
            ident = state["ident"]
            zp = state["zp"]
            protoT2 = state["protoT2"]

            # --- pooling: a[b, d] = (1/T) sum_t hs[b, t, d], in PSUM
            a_ps = psum_a.tile([P, D], f32, tag="a_ps", name="a_ps")
            dma_eng = {"alt2": [nc.sync, nc.scalar],
                       "mono": [nc.sync, nc.sync],
                       "tri": [nc.sync, nc.scalar, nc.gpsimd]}[ring_mode]
            for it in range(NT):
                tl = hs_pool.tile([P, wpp], f32, tag="hs", name="tl")
                src = hs_flat[it * words_per_tile:(it + 1) * words_per_tile]
                s2 = src.rearrange("(p w) -> p w", p=P)
                if split_dma:
                    # both HWDGE rings busy every tile: each ring moves a
                    # contiguous half (partition-split keeps linearity)
                    nc.sync.dma_start(tl[0:P // 2, :], s2[0:P // 2, :])
                    nc.scalar.dma_start(tl[P // 2:P, :], s2[P // 2:P, :])
                else:
                    dma_eng[it % len(dma_eng)].dma_start(tl[:], s2)
                s_i = (it * bpt_num) // bpt_den
                lhsT = zp[:, P - s_i:2 * P - s_i]
                if stage1 and n_rows > 1:
                    partial = part_pool.tile([P, D], f32, tag="part",
                                             name="partial")
                    nc.vector.tensor_add(partial[:], tl[:, 0:D], tl[:, D:2 * D])
                    for j in range(2, n_rows):
                        nc.vector.tensor_add(partial[:], partial[:],
                                             tl[:, j * D:(j + 1) * D])
                    for h in range(2):
                        nc.tensor.matmul(a_ps[:, h * 512:(h + 1) * 512], lhsT,
                                         partial[:, h * 512:(h + 1) * 512],
                                         start=(it == 0), stop=(it == NT - 1),
                                         skip_group_check=True)
                else:
                    # PE consumes raw t-rows directly; all rows of a tile
                    # share the same mask column (same batch coverage)
                    for r in range(n_rows):
                        for h in range(2):
                            nc.tensor.matmul(
                                a_ps[:, h * 512:(h + 1) * 512], lhsT,
                                tl[:, r * D + h * 512:r * D + (h + 1) * 512],
                                start=(it == 0 and r == 0),
                                stop=(it == NT - 1 and r == n_rows - 1),
                                skip_group_check=True)

            a_sb = work.tile([P, D], f32, tag="a", name="a_sb")
            nc.scalar.mul(a_sb[:], a_ps[:], 1.0)

            # a_sq[b] = sum_d a^2 as per-partition scalar [128, 1]
            sq_tmp = work.tile([P, D], f32, tag="sq_tmp", name="sq_tmp")
            asq = work.tile([P, 1], f32, tag="asq", name="asq")
            if ttr_asq:
                nc.vector.tensor_tensor_reduce(
                    out=sq_tmp[:], in0=a_sb[:], in1=a_sb[:], scale=1.0,
                    scalar=0.0, op0=mybir.AluOpType.mult,
                    op1=mybir.AluOpType.add, accum_out=asq[:])
            else:
                nc.vector.tensor_mul(sq_tmp[:], a_sb[:], a_sb[:])
                nc.vector.tensor_reduce(asq[:], sq_tmp[:],
                                        axis=mybir.AxisListType.X,
                                        op=mybir.AluOpType.add)

            # aT[k] = a.T d-chunk [128d, 128b]
            aTs = []
            for k in range(KD):
                pt = psum_t.tile([P, P], f32, tag="tp", name="pt")
                nc.tensor.transpose(pt[:], a_sb[:, k * P:(k + 1) * P], ident[:])
                aT = work.tile([P, P], f32, tag=f"aT{k}", name=f"aT{k}")
                nc.vector.tensor_copy(aT[:], pt[:])
                aTs.append(aT)

            # logits_pre[b, m] = 2 a@proto.T - b_sq in one PSUM bank
            lg_ps = psum_a.tile([P, M_PROTO], f32, tag="lg", name="lg_ps")
            for k in range(KD):
                nc.tensor.matmul(lg_ps[:], aTs[k][:], protoT2[k][:],
                                 start=(k == 0), stop=False)
            nc.tensor.matmul(lg_ps[:], state["ones_k1"][:], state["neg_bsq"][:],
                             start=False, stop=True)

            # subtract a_sq (per-partition scalar broadcast along free dim)
            lg_sb = work.tile([P, M_PROTO], f32, tag="lg_sb", name="lg_sb")
            nc.vector.tensor_scalar_sub(lg_sb[:], lg_ps[:], asq[:])

            # label mean: out[b, l] = mean_p logits_pre[b, p*8 + l]
            out_sb = work.tile([P, NUM_LABELS], f32, tag="out_sb", name="out_sb")
            lgv = lg_sb[:].rearrange("b (p l) -> b l p", l=NUM_LABELS)
            nc.vector.tensor_reduce(out_sb[:], lgv, axis=mybir.AxisListType.X,
                                    op=mybir.AluOpType.add)
            nc.scalar.mul(out_sb[:], out_sb[:], 1.0 / NUM_PROTOTYPES)
            nc.gpsimd.dma_start(out[:, :], out_sb[:])

        if v2:
            prep2()
        else:
            prep()
        body = stream_dma_only if dma_only else (stream2 if v2 else stream)
        if reps == 1:
            body()
        else:
            hints = (mybir.EngineType.DVE, mybir.EngineType.PE,
                     mybir.EngineType.Activation, mybir.EngineType.SP,
                     mybir.EngineType.Pool)
            with tc.For_i(0, reps, 1, hint_engines=hints):
                body()

    nc.compile()
    return nc


def _get_program(reps=1, **kw):
    key = (reps, tuple(sorted(kw.items())))
    if key not in _cached:
        _cached[key] = _build_program(reps, **kw)
    return _cached[key]


def _make_in_maps(hs, pw):
    return [
        {
            "hidden_states": np.ascontiguousarray(hs[i * BS:(i + 1) * BS]),
            "prototype_weight": pw,
        }
        for i in range(N_CORES)
    ]


def run(hidden_states, prototype_weight, trace=False, reps=1):
    """Run the SPMD kernel; returns (full_output, BassKernelResults)."""
    from concourse.bass_utils import run_bass_kernel_spmd

    hs = np.ascontiguousarray(np.asarray(hidden_states, dtype=np.float32))
    pw = np.ascontiguousarray(np.asarray(prototype_weight, dtype=np.float32))
    assert hs.shape == (B, T, D), hs.shape
    assert pw.shape == (M_PROTO, D), pw.shape

    nc = _get_program(reps)
    res = run_bass_kernel_spmd(nc, _make_in_maps(hs, pw),
                               core_ids=list(range(N_CORES)), trace=trace)
    full = np.concatenate([res.results[i]["out"] for i in range(N_CORES)], axis=0)
    return full, res


def kernel(hidden_states, prototype_weight):
    full, _ = run(hidden_states, prototype_weight, trace=False)
    return full



# revision 12
# speedup vs baseline: 1.0420x; 1.0420x over previous
"""Trainium2 Bass kernel for nn_AutoGraderPrototypeModel (retrieval_knn).

Computes, for full inputs hidden_states [1024, 256, 1024] f32 and
prototype_weight [512, 1024] f32:

    a      = mean(hidden_states, axis=1)                  # [B, D]
    logits = 2 a @ proto.T - ||a||^2 - ||proto||^2        # [B, 512]
    out    = logits.reshape(B, 64, 8).mean(axis=1)        # [B, 8]

Sharding: data-parallel over batch across 8 NeuronCores (128 batch rows
per core, prototype table replicated). The dominant cost is streaming the
128 MiB hidden_states shard from HBM; the pure-DMA wall measured on this
part is ~382 us with loop unrolling (349 GB/s/core, ~2.8 TB/s aggregate
— the device HBM limit; per-core bandwidth arbitration is uneven and the
slow-core set varies run to run, so static load balancing does not help).
The kernel runs AT that wall (~380-384 us, ~3 us run-to-run noise).

Loop structure: the For_i hardware loop fully quiesces the pipeline at
every iteration boundary (cross-engine DRAIN barrier + semaphore reset;
~15 us of DMA silence per boundary: ~10 us epilogue + ~5 us drain).
Therefore 16 reps are unrolled inside each For_i iteration so tile pools
stream straight across rep boundaries, and the [128,1024] PSUM
accumulator is double-buffered (a_bufs=2, its own pool opened after the
prep-scoped PSUM pool closes) so the next rep's accumulation overlaps
the epilogue. unroll=32 regresses (397 us); 8 gives ~385.

DMA layout: strided partition reads (1 MiB partition stride) only reach
~190 GB/s/core on this part, while fully-linear reads reach ~350 GB/s.
Tiles are therefore loaded as flat contiguous [128, WPP] blocks, 1 MiB
per dma_start, alternating between the two HWDGE rings (SP/ACT) so the
per-tile buffer-free waits and descriptor generation pipeline in
parallel; with an even buffer count each buffer's reuse stays on one
ring, so WAW ordering rides the ring FIFO.

Compute (v2 path): per tile the DVE folds the two in-partition t-rows
into a bf16 partial; a PE matmul with a sliding block-column bf16 mask
(value 1/T) scatter-accumulates each batch's partitions into PSUM
a[128b, 1024d]. The epilogue transposes a in bf16, runs the 8 logits
matmuls in bf16 against the bf16 2*proto.T table, subtracts ||a||^2 as a
per-partition scalar, label-means on the DVE, and adds the label-meaned
-||proto||^2 as a precomputed [128, 8] broadcast (computed once in prep,
f32). bf16 is safe here: logits are dominated by the exactly-computed
-||proto||^2 term (~1024) and the harness gate is 2e-2; measured rel err
is ~6e-6. NOTE: DVE tensor_tensor_reduce hangs this device — keep
use_ttr=False.
"""

import os

os.environ.setdefault("JAX_PLATFORMS", "axon,cpu")

from contextlib import ExitStack

import numpy as np

B, T, D = 1024, 256, 1024
M_PROTO = 512
NUM_LABELS = 8
NUM_PROTOTYPES = 64
N_CORES = 8
BS = B // N_CORES  # 128 batch rows per core
P = 128            # SBUF partitions
WPP = 2048         # words per partition per DMA tile (tile = P*WPP*4 bytes)
HS_BUFS = 10

_cached = {}


def _build_program(reps=1, wpp=WPP, hs_bufs=HS_BUFS, act_pt2=False,
                   ttr_asq=False, stage1=True, split_dma=False,
                   dma_only=False, ring_mode="alt2", v2=True,
                   part_bufs=4, a_bufs=2, pool_dt="bf16", epi_dt="bf16",
                   use_ttr=False, alt_consumer=False, unroll=16,
                   epi_bufs=2):
    import math

    import concourse.mybir as mybir
    import concourse.tile as tile
    from concourse import bacc, masks

    unroll = math.gcd(unroll, reps)

    f32 = mybir.dt.float32
    bf16 = mybir.dt.bfloat16
    pdt = bf16 if pool_dt == "bf16" else f32
    edt = bf16 if epi_dt == "bf16" else f32
    KD = D // P                      # 8 contraction chunks of 128 over D
    MG = M_PROTO // P                # 4 prototype groups of 128
    words_per_tile = P * wpp
    NT = (BS * T * D) // words_per_tile  # linear tiles per shard
    n_rows = wpp // D                # t-rows per partition (stage-1 depth)
    assert wpp % D == 0
    # batches per tile as a fraction: bpt_num/bpt_den
    bpt_num, bpt_den = words_per_tile, T * D
    n_cols = max(bpt_num // bpt_den, 1)   # mask columns per tile
    grp = P // n_cols if bpt_num >= bpt_den else P

    nc = bacc.Bacc("TRN2", target_bir_lowering=False, debug=False,
                   num_devices=N_CORES)
    hs = nc.dram_tensor("hidden_states", [BS, T, D], f32, kind="ExternalInput").ap()
    pw = nc.dram_tensor("prototype_weight", [M_PROTO, D], f32, kind="ExternalInput").ap()
    out = nc.dram_tensor("out", [BS, NUM_LABELS], f32, kind="ExternalOutput").ap()

    hs_flat = hs.rearrange("b t d -> (b t d)")

    with tile.TileContext(nc) as tc, ExitStack() as ctx:
        hs_pool = ctx.enter_context(tc.tile_pool(name="hs", bufs=hs_bufs))
        part_pool = ctx.enter_context(tc.tile_pool(name="part", bufs=part_bufs))
        work = ctx.enter_context(tc.tile_pool(name="work", bufs=1))
        epi = ctx.enter_context(tc.tile_pool(name="epi", bufs=epi_bufs))
        psum_t = ctx.enter_context(tc.tile_pool(name="psum_t", bufs=2, space="PSUM"))
        psum_a = ctx.enter_context(tc.tile_pool(name="psum_a", bufs=1, space="PSUM"))

        state = {}

        def prep2(psum_prep):
            # bf16 identity for a-transposes; zp mask exact in bf16 (1/256)
            ident = work.tile([P, P], f32, tag="ident", name="ident")
            masks.make_identity(nc, ident[:])
            ident_bf = work.tile([P, P], edt, tag="ident_bf", name="ident_bf")
            masks.make_identity(nc, ident_bf[:])
            ones_m1 = work.tile([P, 1], f32, tag="ones_m1", name="ones_m1")
            nc.gpsimd.memset(ones_m1[:], 1.0)
            ones_k1 = work.tile([1, P], f32, tag="ones_k1", name="ones_k1")
            nc.gpsimd.memset(ones_k1[:], 1.0)

            zp = work.tile([P, 2 * P], pdt, tag="zp", name="zp")
            nc.gpsimd.memset(zp[:], 0.0)
            for c in range(n_cols):
                nc.gpsimd.memset(zp[grp * c:grp * (c + 1), P + c:P + c + 1],
                                 1.0 / T)

            proto_sb = []
            for j in range(MG):
                pj = work.tile([P, D], f32, tag=f"proto{j}", name=f"proto{j}")
                nc.gpsimd.dma_start(pj[:], pw[j * P:(j + 1) * P, :])
                proto_sb.append(pj)

            # protoT2_bf[k] = bf16(2 proto.T chunk); b_sq accumulated in f32
            protoT2 = [work.tile([P, M_PROTO], edt, tag=f"pT2b_{k}",
                                 name=f"pT2b_{k}") for k in range(KD)]
            p2f = work.tile([P, M_PROTO], f32, tag="p2f", name="p2f")
            sqT = work.tile([P, M_PROTO], f32, tag="sqT", name="sqT")
            bsq_ps = psum_prep.tile([1, M_PROTO], f32, tag="bsq", name="bsq_ps")
            for k in range(KD):
                for j in range(MG):
                    pt = psum_prep.tile([P, P], f32, tag="tp", name="pt", bufs=1)
                    nc.tensor.transpose(pt[:], proto_sb[j][:, k * P:(k + 1) * P],
                                        ident[:])
                    nc.vector.tensor_scalar_mul(
                        p2f[:, j * P:(j + 1) * P], pt[:], 2.0)
                nc.vector.tensor_copy(protoT2[k][:], p2f[:])
                nc.vector.tensor_mul(sqT[:], p2f[:], p2f[:])
                nc.tensor.matmul(bsq_ps[:], ones_m1[:], sqT[:],
                                 start=(k == 0), stop=(k == KD - 1))
            # nbl[l] = -(1/64) sum_p b_sq[p*8+l]; (2p)^2 scale fixed by -0.25
            neg_bsq = work.tile([1, M_PROTO], f32, tag="neg_bsq", name="neg_bsq")
            nc.scalar.mul(neg_bsq[:], bsq_ps[:], -0.25)
            nbl = work.tile([1, NUM_LABELS], f32, tag="nbl", name="nbl")
            nbv = neg_bsq[:].rearrange("a (p l) -> a l p", l=NUM_LABELS)
            nc.vector.tensor_reduce(nbl[:], nbv, axis=mybir.AxisListType.X,
                                    op=mybir.AluOpType.add)
            nc.scalar.mul(nbl[:], nbl[:], 1.0 / NUM_PROTOTYPES)
            ptb = psum_prep.tile([P, P], f32, tag="tp", name="ptb", bufs=1)
            nc.tensor.matmul(ptb[:, 0:NUM_LABELS], ones_k1[:], nbl[:],
                             start=True, stop=True)
            nbl_bcast = work.tile([P, NUM_LABELS], f32, tag="nbl_b",
                                  name="nbl_bcast")
            nc.vector.tensor_copy(nbl_bcast[:], ptb[:, 0:NUM_LABELS])

            state.update(ident_bf=ident_bf, zp=zp, protoT2=protoT2,
                         nbl_bcast=nbl_bcast)

        def stream2():
            zp = state["zp"]
            protoT2 = state["protoT2"]
            dma_eng = {"alt2": [nc.sync, nc.scalar],
                       "mono": [nc.sync, nc.sync],
                       "tri": [nc.sync, nc.scalar, nc.gpsimd]}[ring_mode]
            ne = len(dma_eng)

            a_ps = psum_acc.tile([P, D], f32, tag="a_ps", name="a_ps")
            for it in range(NT):
                tl = hs_pool.tile([P, wpp], f32, tag="hs", name="tl")
                src = hs_flat[it * words_per_tile:(it + 1) * words_per_tile]
                s2 = src.rearrange("(p w) -> p w", p=P)
                dma_eng[it % ne].dma_start(tl[:], s2)
                s_i = (it * bpt_num) // bpt_den
                lhsT = zp[:, P - s_i:2 * P - s_i]
                partial = part_pool.tile([P, D], pdt, tag="part",
                                         name="partial")
                # stage-1 adds on DVE, or alternating DVE/Pool per tile
                veng = nc.gpsimd if (alt_consumer and it % 2) else nc.vector
                if n_rows == 2:
                    veng.tensor_add(partial[:], tl[:, 0:D], tl[:, D:2 * D])
                else:
                    # chain f32 adds, final add casts to pool dtype
                    pf = part_pool.tile([P, D], f32, tag="partf", name="pf")
                    veng.tensor_add(pf[:], tl[:, 0:D], tl[:, D:2 * D])
                    for j in range(2, n_rows - 1):
                        veng.tensor_add(pf[:], pf[:],
                                        tl[:, j * D:(j + 1) * D])
                    veng.tensor_add(partial[:], pf[:],
                                    tl[:, (n_rows - 1) * D:n_rows * D])
                for h in range(2):
                    nc.tensor.matmul(a_ps[:, h * 512:(h + 1) * 512], lhsT,
                                     partial[:, h * 512:(h + 1) * 512],
                                     start=(it == 0), stop=(it == NT - 1),
                                     skip_group_check=True)

            # epilogue: a in bf16, logits matmuls in bf16, b_sq folded post-hoc
            a_sb = epi.tile([P, D], edt, tag="a", name="a_sb")
            nc.scalar.mul(a_sb[:], a_ps[:], 1.0)

            sq_tmp = epi.tile([P, D], f32, tag="sq_tmp", name="sq_tmp")
            asq = epi.tile([P, 1], f32, tag="asq", name="asq")
            if use_ttr:
                nc.vector.tensor_tensor_reduce(
                    out=sq_tmp[:], in0=a_sb[:], in1=a_sb[:], scale=1.0,
                    scalar=0.0, op0=mybir.AluOpType.mult,
                    op1=mybir.AluOpType.add, accum_out=asq[:])
            else:
                nc.vector.tensor_mul(sq_tmp[:], a_sb[:], a_sb[:])
                nc.vector.tensor_reduce(asq[:], sq_tmp[:],
                                        axis=mybir.AxisListType.X,
                                        op=mybir.AluOpType.add)

            aTs = []
            for k in range(KD):
                pt = psum_t.tile([P, P], edt, tag="tpb", name="pt")
                nc.tensor.transpose(pt[:], a_sb[:, k * P:(k + 1) * P],
                                    state["ident_bf"][:])
                aT = epi.tile([P, P], edt, tag=f"aT{k}", name=f"aT{k}")
                nc.vector.tensor_copy(aT[:], pt[:])
                aTs.append(aT)

            lg_ps = psum_a.tile([P, M_PROTO], f32, tag="lg", name="lg_ps")
            for k in range(KD):
                nc.tensor.matmul(lg_ps[:], aTs[k][:], protoT2[k][:],
                                 start=(k == 0), stop=(k == KD - 1))

            lg_sb = epi.tile([P, M_PROTO], f32, tag="lg_sb", name="lg_sb")
            nc.vector.tensor_scalar_sub(lg_sb[:], lg_ps[:], asq[:])

            out_pre = epi.tile([P, NUM_LABELS], f32, tag="out_pre",
                               name="out_pre")
            lgv = lg_sb[:].rearrange("b (p l) -> b l p", l=NUM_LABELS)
            nc.vector.tensor_reduce(out_pre[:], lgv, axis=mybir.AxisListType.X,
                                    op=mybir.AluOpType.add)
            nc.scalar.mul(out_pre[:], out_pre[:], 1.0 / NUM_PROTOTYPES)
            out_sb = epi.tile([P, NUM_LABELS], f32, tag="out_sb", name="out_sb")
            nc.vector.tensor_add(out_sb[:], out_pre[:], state["nbl_bcast"][:])
            nc.gpsimd.dma_start(out[:, :], out_sb[:])

        def prep(psum_prep):
            ident = work.tile([P, P], f32, tag="ident", name="ident")
            masks.make_identity(nc, ident[:])
            ones_m1 = work.tile([P, 1], f32, tag="ones_m1", name="ones_m1")
            nc.gpsimd.memset(ones_m1[:], 1.0)
            ones_k1 = work.tile([1, P], f32, tag="ones_k1", name="ones_k1")
            nc.gpsimd.memset(ones_k1[:], 1.0)

            # Sliding mask for stage-2 pooling: zp[p, P + c] = 1/T iff
            # c == p // grp (c < n_cols). lhsT for tile i is
            # zp[:, P - s_i : 2P - s_i] with s_i = floor(i * bpt).
            zp = work.tile([P, 2 * P], f32, tag="zp", name="zp")
            nc.gpsimd.memset(zp[:], 0.0)
            for c in range(n_cols):
                nc.gpsimd.memset(zp[grp * c:grp * (c + 1), P + c:P + c + 1],
                                 1.0 / T)

            # protoT2[k] = 2 * proto.T d-chunk; sqT[k] = (2 proto.T)^2
            proto_sb = []
            for j in range(MG):
                pj = work.tile([P, D], f32, tag=f"proto{j}", name=f"proto{j}")
                nc.gpsimd.dma_start(pj[:], pw[j * P:(j + 1) * P, :])
                proto_sb.append(pj)

            protoT2 = [work.tile([P, M_PROTO], f32, tag=f"pT2_{k}", name=f"pT2_{k}")
                       for k in range(KD)]
            sqT = [work.tile([P, M_PROTO], f32, tag=f"sqT_{k}", name=f"sqT_{k}")
                   for k in range(KD)]
            for k in range(KD):
                for j in range(MG):
                    pt = psum_prep.tile([P, P], f32, tag="tp", name="pt")
                    nc.tensor.transpose(pt[:], proto_sb[j][:, k * P:(k + 1) * P],
                                        ident[:])
                    if act_pt2:
                        nc.scalar.mul(protoT2[k][:, j * P:(j + 1) * P],
                                      pt[:], 2.0)
                    else:
                        nc.vector.tensor_scalar_mul(
                            protoT2[k][:, j * P:(j + 1) * P], pt[:], 2.0)
                # (2 protoT)^2 = 4 protoT^2; compensated below via -0.25 scale
                nc.vector.tensor_mul(sqT[k][:], protoT2[k][:], protoT2[k][:])

            # b_sq[m] as a [1, 512] row via ones-matmul over squared protoT
            bsq_ps = psum_prep.tile([1, M_PROTO], f32, tag="bsq", name="bsq_ps")
            for k in range(KD):
                nc.tensor.matmul(bsq_ps[:], ones_m1[:], sqT[k][:],
                                 start=(k == 0), stop=(k == KD - 1))
            neg_bsq = work.tile([1, M_PROTO], f32, tag="neg_bsq", name="neg_bsq")
            nc.scalar.mul(neg_bsq[:], bsq_ps[:], -0.25)

            state.update(ident=ident, ones_k1=ones_k1, zp=zp, neg_bsq=neg_bsq,
                         protoT2=protoT2)

        def stream_dma_only():
            # BW probe: stream all tiles, no compute consumers (WAW deps on
            # the pool bufs still order reuse); write a dummy output.
            dma_eng = {"alt2": [nc.sync, nc.scalar],
                       "mono": [nc.sync, nc.sync],
                       "tri": [nc.sync, nc.scalar, nc.gpsimd]}[ring_mode]
            ne = len(dma_eng)
            for it in range(NT):
                tl = hs_pool.tile([P, wpp], f32, tag="hs", name="tl")
                src = hs_flat[it * words_per_tile:(it + 1) * words_per_tile]
                s2 = src.rearrange("(p w) -> p w", p=P)
                dma_eng[it % ne].dma_start(tl[:], s2)
            out_sb = work.tile([P, NUM_LABELS], f32, tag="out_sb",
                               name="out_sb")
            nc.gpsimd.memset(out_sb[:], 0.0)
            nc.gpsimd.dma_start(out[:, :], out_sb[:])

        def stream():
            ident = state["ident"]
            zp = state["zp"]
            protoT2 = state["protoT2"]

            # --- pooling: a[b, d] = (1/T) sum_t hs[b, t, d], in PSUM
            a_ps = psum_acc.tile([P, D], f32, tag="a_ps", name="a_ps")
            dma_eng = {"alt2": [nc.sync, nc.scalar],
                       "mono": [nc.sync, nc.sync],
                       "tri": [nc.sync, nc.scalar, nc.gpsimd]}[ring_mode]
            for it in range(NT):
                tl = hs_pool.tile([P, wpp], f32, tag="hs", name="tl")
                src = hs_flat[it * words_per_tile:(it + 1) * words_per_tile]
                s2 = src.rearrange("(p w) -> p w", p=P)
                if split_dma:
                    # both HWDGE rings busy every tile: each ring moves a
                    # contiguous half (partition-split keeps linearity)
                    nc.sync.dma_start(tl[0:P // 2, :], s2[0:P // 2, :])
                    nc.scalar.dma_start(tl[P // 2:P, :], s2[P // 2:P, :])
                else:
                    dma_eng[it % len(dma_eng)].dma_start(tl[:], s2)
                s_i = (it * bpt_num) // bpt_den
                lhsT = zp[:, P - s_i:2 * P - s_i]
                if stage1 and n_rows > 1:
                    partial = part_pool.tile([P, D], f32, tag="part",
                                             name="partial")
                    nc.vector.tensor_add(partial[:], tl[:, 0:D], tl[:, D:2 * D])
                    for j in range(2, n_rows):
                        nc.vector.tensor_add(partial[:], partial[:],
                                             tl[:, j * D:(j + 1) * D])
                    for h in range(2):
                        nc.tensor.matmul(a_ps[:, h * 512:(h + 1) * 512], lhsT,
                                         partial[:, h * 512:(h + 1) * 512],
                                         start=(it == 0), stop=(it == NT - 1),
                                         skip_group_check=True)
                else:
                    # PE consumes raw t-rows directly; all rows of a tile
                    # share the same mask column (same batch coverage)
                    for r in range(n_rows):
                        for h in range(2):
                            nc.tensor.matmul(
                                a_ps[:, h * 512:(h + 1) * 512], lhsT,
                                tl[:, r * D + h * 512:r * D + (h + 1) * 512],
                                start=(it == 0 and r == 0),
                                stop=(it == NT - 1 and r == n_rows - 1),
                                skip_group_check=True)

            a_sb = work.tile([P, D], f32, tag="a", name="a_sb")
            nc.scalar.mul(a_sb[:], a_ps[:], 1.0)

            # a_sq[b] = sum_d a^2 as per-partition scalar [128, 1]
            sq_tmp = work.tile([P, D], f32, tag="sq_tmp", name="sq_tmp")
            asq = work.tile([P, 1], f32, tag="asq", name="asq")
            if ttr_asq:
                nc.vector.tensor_tensor_reduce(
                    out=sq_tmp[:], in0=a_sb[:], in1=a_sb[:], scale=1.0,
                    scalar=0.0, op0=mybir.AluOpType.mult,
                    op1=mybir.AluOpType.add, accum_out=asq[:])
            else:
                nc.vector.tensor_mul(sq_tmp[:], a_sb[:], a_sb[:])
                nc.vector.tensor_reduce(asq[:], sq_tmp[:],
                                        axis=mybir.AxisListType.X,
                                        op=mybir.AluOpType.add)

            # aT[k] = a.T d-chunk [128d, 128b]
            aTs = []
            for k in range(KD):
                pt = psum_t.tile([P, P], f32, tag="tp", name="pt")
                nc.tensor.transpose(pt[:], a_sb[:, k * P:(k + 1) * P], ident[:])
                aT = work.tile([P, P], f32, tag=f"aT{k}", name=f"aT{k}")
                nc.vector.tensor_copy(aT[:], pt[:])
                aTs.append(aT)

            # logits_pre[b, m] = 2 a@proto.T - b_sq in one PSUM bank
            lg_ps = psum_a.tile([P, M_PROTO], f32, tag="lg", name="lg_ps")
            for k in range(KD):
                nc.tensor.matmul(lg_ps[:], aTs[k][:], protoT2[k][:],
                                 start=(k == 0), stop=False)
            nc.tensor.matmul(lg_ps[:], state["ones_k1"][:], state["neg_bsq"][:],
                             start=False, stop=True)

            # subtract a_sq (per-partition scalar broadcast along free dim)
            lg_sb = work.tile([P, M_PROTO], f32, tag="lg_sb", name="lg_sb")
            nc.vector.tensor_scalar_sub(lg_sb[:], lg_ps[:], asq[:])

            # label mean: out[b, l] = mean_p logits_pre[b, p*8 + l]
            out_sb = work.tile([P, NUM_LABELS], f32, tag="out_sb", name="out_sb")
            lgv = lg_sb[:].rearrange("b (p l) -> b l p", l=NUM_LABELS)
            nc.vector.tensor_reduce(out_sb[:], lgv, axis=mybir.AxisListType.X,
                                    op=mybir.AluOpType.add)
            nc.scalar.mul(out_sb[:], out_sb[:], 1.0 / NUM_PROTOTYPES)
            nc.gpsimd.dma_start(out[:, :], out_sb[:])

        with tc.tile_pool(name="psum_prep", bufs=1, space="PSUM") as pp:
            if v2:
                prep2(pp)
            else:
                prep(pp)
        psum_acc = ctx.enter_context(
            tc.tile_pool(name="psum_acc", bufs=a_bufs, space="PSUM"))
        body = stream_dma_only if dma_only else (stream2 if v2 else stream)
        # Unroll `unroll` reps inside each For_i iteration: the hardware
        # loop's per-iteration cross-engine DRAIN barrier + semaphore
        # reset fully quiesces the DMA pipeline (~15 us measured), so
        # amortize it over `unroll` reps; tile pools pipeline freely
        # across the unrolled rep boundaries.
        assert reps % unroll == 0, (reps, unroll)
        n_iter = reps // unroll
        if n_iter == 1:
            for _ in range(unroll):
                body()
        else:
            hints = (mybir.EngineType.DVE, mybir.EngineType.PE,
                     mybir.EngineType.Activation, mybir.EngineType.SP,
                     mybir.EngineType.Pool)
            with tc.For_i(0, n_iter, 1, hint_engines=hints):
                for _ in range(unroll):
                    body()

    nc.compile()
    return nc


def _get_program(reps=1, **kw):
    key = (reps, tuple(sorted(kw.items())))
    if key not in _cached:
        _cached[key] = _build_program(reps, **kw)
    return _cached[key]


def _make_in_maps(hs, pw):
    return [
        {
            "hidden_states": np.ascontiguousarray(hs[i * BS:(i + 1) * BS]),
            "prototype_weight": pw,
        }
        for i in range(N_CORES)
    ]


def run(hidden_states, prototype_weight, trace=False, reps=1):
    """Run the SPMD kernel; returns (full_output, BassKernelResults)."""
    from concourse.bass_utils import run_bass_kernel_spmd

    hs = np.ascontiguousarray(np.asarray(hidden_states, dtype=np.float32))
    pw = np.ascontiguousarray(np.asarray(prototype_weight, dtype=np.float32))
    assert hs.shape == (B, T, D), hs.shape
    assert pw.shape == (M_PROTO, D), pw.shape

    nc = _get_program(reps)
    res = run_bass_kernel_spmd(nc, _make_in_maps(hs, pw),
                               core_ids=list(range(N_CORES)), trace=trace)
    full = np.concatenate([res.results[i]["out"] for i in range(N_CORES)], axis=0)
    return full, res


def kernel(hidden_states, prototype_weight):
    full, _ = run(hidden_states, prototype_weight, trace=False)
    return full

